# revision 36
# baseline (speedup 1.0000x reference)
"""Trainium2 Bass kernel for nn_Attention_36644660969693.

Multi-head attention block: x[8,32,32,768] -> qkv -> 12-head attention -> wo.
Sharding: data-parallel over batch, one image (1024 tokens) per NeuronCore.

Per-core layout strategy (T=1024 tokens, C=768, 12 heads, hd=64):
  - all inputs arrive via gpsimd (SWDGE) casting DMAs straight into fp16
    SBUF tiles, column-sliced in priority order: x, pair-0 q/k columns,
    v columns, remaining q/k columns, w_o
  - xT[c,t] via PE transpose of x (fp16)
  - qkT[f,t] = w_qkv tile-stationary @ xT; head h lands at partition
    (h*64)%128 of f-tile h//2, so a head PAIR occupies the two partition
    halves of one tile -> 2-head row-packed score matmuls
  - v[t,f] natural orientation, stored per pair as [v_even | ones | v_odd]
    (192 cols); the AV stationary [128,128] slice for the even head is
    [v|1] and for the odd head [1|v], so each AV matmul also emits the
    softmax row-sums in the complementary 64 output partitions for free;
    v is computed just-in-time inside pair 0's loop
  - scoresT[j,i] per head = kT-tile-stationary @ qT (K=64, two heads packed
    into disjoint PE row groups), exp on ScalarE reads PSUM directly with
    the 1/8 scale fused, output fp16 to SBUF
  - AV accumulates v_pad.T @ expT over key tiles; 3 of 4 head/chunk
    accumulations pipeline inside the score/exp loop (lagging exp), the 4th
    is deferred into the next pair's loop so ScalarE never waits at a pair
    boundary
  - normalize = full-partition fast-reciprocal of the row-sum half +
    64-partition swap DMA + elementwise multiply
  - out[t,:] = attn_T-tile-stationary @ w_o (natural layout, DMA straight
    out)
All matmuls run in fp16 (1 cycle/row on PE) with fp32 PSUM accumulation.
ScalarE exp (~17us/head-pair) is the pipeline bottleneck; PE work is
interleaved under it.
"""

import numpy as np

import concourse.bass as bass
import concourse.tile as tile
from concourse import bacc, mybir
from concourse import bass_utils
from concourse import masks

P = 128          # partitions
T = 1024         # tokens per image
C = 768          # model dim
NT = T // P      # 8 token tiles
NC = C // P      # 6 channel tiles
NH = 12          # heads
HD = 64          # head dim
NPAIR = NH // 2  # 6 head pairs
VPW = 192        # v_pad pair block width: [v_even(64) | ones(64) | v_odd(64)]
SCALE = HD ** -0.5
F32 = mybir.dt.float32
F16 = mybir.dt.float16
EXP = mybir.ActivationFunctionType.Exp


def attention_kernel(tc, out_d, x_d, wq_d, wo_d):
    nc = tc.nc
    from contextlib import ExitStack

    with ExitStack() as ctx:
        const_pool = ctx.enter_context(tc.tile_pool(name="const", bufs=1))
        persist = ctx.enter_context(tc.tile_pool(name="persist", bufs=1))
        opool = ctx.enter_context(tc.tile_pool(name="ot", bufs=2))

        identh = const_pool.tile([P, P], F16, tag="identh")
        masks.make_identity(nc, identh[:])

        xT = persist.tile([P, NC * T], F16, tag="xT")        # [c, t] blocks
        wq = persist.tile([P, NC * 2304], F16, tag="wq")     # [c, f] blocks
        qkT = persist.tile([P, 12 * T], F16, tag="qkT")      # [f, t] blocks
        vpad = persist.tile([P, NT * NPAIR * VPW], F16, tag="vpad")
        aT = persist.tile([P, NC * T], F16, tag="aT")        # [c, t] blocks
        wo_sb = persist.tile([P, NC * C], F16, tag="wo")     # [c, c'] blocks

        # ones blocks of v_pad: cols 64:128 of each 192-col pair block
        ones_ap = vpad[:].rearrange(
            "p (blk w) -> p blk w", w=VPW
        )[:, :, HD: 2 * HD]
        nc.vector.memset(ones_ap, 1.0)

        # ---- input DMAs: gpsimd SWDGE casts f32->f16 in flight ----
        # priority order: x, pair-0 q/k columns, v columns, bulk q/k, w_o
        qkcol = [[768, 2], [1, 128]]     # cols 0:128 and 768:896
        bulkcol = [[768, 2], [1, 640]]   # cols 128:768 and 896:1536

        def wq_col_dma(ct, pattern, off):
            dst = wq[:, ct * 2304 + off: (ct + 1) * 2304]
            dst = bass.AP(
                tensor=dst.tensor, offset=dst.offset, ap=[dst.ap[0]] + pattern)
            s = wq_d[ct * P:(ct + 1) * P, off:2304]
            src = bass.AP(
                tensor=s.tensor, offset=s.offset, ap=[s.ap[0]] + pattern)
            nc.gpsimd.dma_start(dst, src)

        prep_ctx = ExitStack()
        pp_prep = prep_ctx.enter_context(
            tc.tile_pool(name="pprep", bufs=2, space="PSUM"))

        with tc.tile_pool(name="sx", bufs=2) as stage_x:
            for tb in range(2):
                xh = stage_x.tile([P, 4 * C], F16, tag="xh", name="xh")
                src = x_d[tb * 512:(tb + 1) * 512, :].rearrange(
                    "(k p) c -> p k c", p=P
                )
                nc.gpsimd.dma_start(
                    xh[:].rearrange("p (k c) -> p k c", k=4), src
                )
                for ct in range(NC):
                    ps = pp_prep.tile([P, 512], F16, tag="tr", name="ps_tr")
                    for k in range(4):
                        nc.tensor.transpose(
                            ps[:, k * P:(k + 1) * P],
                            xh[:, k * C + ct * P: k * C + ct * P + P],
                            identh[:],
                        )
                    dst_off = ct * T + tb * 512
                    nc.vector.tensor_copy(xT[:, dst_off: dst_off + 512], ps[:])

        for ct in range(NC):
            wq_col_dma(ct, qkcol, 0)          # pair-0 q/k columns
        for ct in range(NC):
            wq_col_dma(ct, [[1, 768]], 1536)  # v columns
        for ct in range(NC):
            wq_col_dma(ct, bulkcol, 128)      # remaining q/k columns
        for ct in range(NC):
            nc.gpsimd.dma_start(wo_sb[:, ct * C:(ct + 1) * C],
                                wo_d[ct * P:(ct + 1) * P, :])

        qk_psum = {"pool": pp_prep, "tag": "mm"}

        def make_qk_emitter(ft):
            """Returns step(n): emits n accumulation matmuls of the qkT
            f-tile computation, so the work interleaves finely with the
            score stream instead of blocking it."""
            pool, tag = qk_psum["pool"], qk_psum["tag"]
            st = {"ch": 0, "ct": 0, "ps": None}

            def step(n):
                for _ in range(n):
                    ch, ct = st["ch"], st["ct"]
                    if ch >= 2:
                        return
                    if ct == 0:
                        st["ps"] = pool.tile([P, 512], F32, tag=tag, name="ps_qk")
                    nc.tensor.matmul(
                        st["ps"][:],
                        wq[:, ct * 2304 + ft * P: ct * 2304 + ft * P + P],
                        xT[:, ct * T + ch * 512: ct * T + ch * 512 + 512],
                        start=(ct == 0),
                        stop=(ct == NC - 1),
                    )
                    if ct == NC - 1:
                        nc.vector.tensor_copy(
                            qkT[:, ft * T + ch * 512: ft * T + ch * 512 + 512],
                            st["ps"][:])
                        st["ch"], st["ct"] = ch + 1, 0
                    else:
                        st["ct"] = ct + 1
            return step

        def emit_qk_tile(ft):
            make_qk_emitter(ft)(2 * NC)

        emit_qk_tile(0)
        emit_qk_tile(6)
        prep_ctx.close()

        # ---- attention, one head pair at a time ----
        epool = ctx.enter_context(tc.tile_pool(name="E", bufs=2))
        rpool = ctx.enter_context(tc.tile_pool(name="recip", bufs=2))
        pp_s = ctx.enter_context(tc.tile_pool(name="pps", bufs=2, space="PSUM"))
        pp_av = ctx.enter_context(tc.tile_pool(name="ppav", bufs=4, space="PSUM"))
        qk_psum["pool"], qk_psum["tag"] = pp_av, "av"

        wo_state = {}

        def emit_wo_half(tt, half, pool=None, tag="av"):
            """Half of one output t-tile projection (6 matmuls)."""
            pool = pool or pp_av
            if half == 0:
                po1 = pool.tile([P, 512], F32, tag=tag, name="po1")
                wo_state[tt] = po1
                for ct in range(NC):
                    lhsT = aT[:, ct * T + tt * P: ct * T + tt * P + P]
                    nc.tensor.matmul(po1[:], lhsT, wo_sb[:, ct * C: ct * C + 512],
                                     start=(ct == 0), stop=(ct == NC - 1))
            else:
                po1 = wo_state.pop(tt)
                po2 = pool.tile([P, 512], F32, tag=tag, name="po2")
                for ct in range(NC):
                    lhsT = aT[:, ct * T + tt * P: ct * T + tt * P + P]
                    nc.tensor.matmul(po2[:, :256], lhsT,
                                     wo_sb[:, ct * C + 512: ct * C + C],
                                     start=(ct == 0), stop=(ct == NC - 1))
                ot = opool.tile([P, C], F32, tag="ot", name="ot")
                nc.scalar.copy(ot[:, 0:512], po1[:])
                nc.sync.dma_start(out_d[tt * P:(tt + 1) * P, 0:512], ot[:, 0:512])
                nc.vector.tensor_copy(ot[:, 512:C], po2[:, :256])
                nc.sync.dma_start(out_d[tt * P:(tt + 1) * P, 512:C], ot[:, 512:C])

        def emit_wo(tts):
            for i, tt in enumerate(tts):
                kw = {} if i % 2 == 0 else {"pool": pp_s, "tag": "s"}
                emit_wo_half(tt, 0, **kw)
                emit_wo_half(tt, 1, **kw)

        def vslice(jt, hp, h):
            """[128,128] AV stationary: even head [v|1], odd head [1|v]."""
            base = jt * NPAIR * VPW + hp * VPW + (0 if h == 0 else HD)
            return vpad[:, base: base + P]

        def emit_v_chunk(tt, chunk):
            """One 6-matmul chunk of v[t-tile tt] (chunk 0: heads 0-7,
            chunk 1: heads 8-11)."""
            for (foff, fw) in ((0, 512), (512, 256))[chunk:chunk + 1]:
                ps = pp_av.tile([P, 512], F32, tag="av", name="ps_v")
                for ct in range(NC):
                    nc.tensor.matmul(
                        ps[:, :fw],
                        xT[:, ct * T + tt * P: ct * T + tt * P + P],
                        wq[:, ct * 2304 + 1536 + foff: ct * 2304 + 1536 + foff + fw],
                        start=(ct == 0),
                        stop=(ct == NC - 1),
                    )
                npr = fw // 128
                src = ps[:, :fw].rearrange("p (m two d) -> p m two d", two=2, d=HD)
                base = tt * NPAIR * VPW + (foff // 128) * VPW
                dst = vpad[:, base: base + npr * VPW].rearrange(
                    "p (m blk) -> p m blk", blk=VPW
                )
                nc.vector.tensor_copy(dst[:, :, 0:HD], src[:, :, 0, :])
                nc.vector.tensor_copy(dst[:, :, 2 * HD:VPW], src[:, :, 1, :])

        def normalize(a, h, hp, ch):
            r = rpool.tile([P, 512], F32, tag="r", name=f"r{h}{ch}")
            r2 = rpool.tile([P, 512], F32, tag="r2", name=f"r2{h}{ch}")
            dst = aT[:, hp * T + ch * 512: hp * T + ch * 512 + 512]
            # full-partition approx reciprocal (custom DVE op needs base
            # partition 0); the non-rowsum half of r is garbage, never read
            nc.vector.reciprocal_approx_fast(r[:, :], a[:, :])
            if h == 0:
                nc.sync.dma_start(r2[0:HD, :], r[HD:P, :])
                nc.vector.tensor_mul(dst[0:HD, :], a[0:HD, :], r2[0:HD, :])
            else:
                nc.sync.dma_start(r2[HD:P, :], r[0:HD, :])
                nc.vector.tensor_mul(dst[HD:P, :], a[HD:P, :], r2[HD:P, :])

        def eoff(jt, ch, h):
            return jt * 2048 + ch * T + h * 512

        def emit_scores_exp(hp, jt, ch, E):
            """Both packed heads' scores for one i-chunk into ONE 2-bank
            psum tile (forces the row-group pair to issue back-to-back),
            then a single exp over the pair."""
            qblk = hp * T
            kblk = (6 + hp) * T
            s = pp_s.tile([P, T], F32, tag="s", name="s")
            nc.tensor.matmul(
                s[:, 0:512],
                qkT[0:HD, kblk + jt * P: kblk + jt * P + P],
                qkT[0:HD, qblk + ch * 512: qblk + ch * 512 + 512],
                start=True, stop=True,
            )
            nc.tensor.matmul(
                s[:, 512:1024],
                qkT[HD:P, kblk + jt * P: kblk + jt * P + P],
                qkT[HD:P, qblk + ch * 512: qblk + ch * 512 + 512],
                start=True, stop=True,
            )
            nc.scalar.activation(E[:, eoff(jt, ch, 0): eoff(jt, ch, 0) + T],
                                 s[:], EXP, scale=SCALE)

        pending_tail = None   # previous half-pass: final AV steps + normalizes

        for hp in range(NPAIR):
            E = epool.tile([P, NT * 2048], F16, tag="E", name="E")
            last = hp == NPAIR - 1

            for ch in range(2):
                a0 = pp_av.tile([P, 512], F32, tag="av", name=f"a0c{ch}")
                a1 = pp_av.tile([P, 512], F32, tag="av", name=f"a1c{ch}")

                def av_step(jt, ch=ch, a0=a0, a1=a1, hp=hp, E=E):
                    for a, h in ((a0, 0), (a1, 1)):
                        nc.tensor.matmul(
                            a[:],
                            vslice(jt, hp, h),
                            E[:, eoff(jt, ch, h): eoff(jt, ch, h) + 512],
                            start=(jt == 0),
                            stop=(jt == NT - 1),
                        )

                # fine-grained filler schedule for this half-pass
                if hp == 0 and ch == 1:
                    qk_steps = [make_qk_emitter(1), make_qk_emitter(7)]
                elif 0 < hp < NPAIR - 1:
                    qk_steps = [make_qk_emitter(hp + 1 if ch == 0 else 6 + hp + 1)]
                else:
                    qk_steps = []

                for jtp in range(0, NT, 2):
                    # two adjacent score-pair groups: their stationaries sit
                    # in disjoint PE row groups, so weight loads pre-overlap
                    for jt in (jtp, jtp + 1):
                        emit_scores_exp(hp, jt, ch, E)
                        if jt == 0 and pending_tail is not None:
                            pending_tail()
                            pending_tail = None
                        if hp == 0 and ch == 0:
                            emit_v_chunk(jt, 0)
                            if jt >= 1:
                                emit_v_chunk(jt - 1, 1)
                    for jt in (jtp, jtp + 1):
                        if jt >= 2:
                            av_step(jt - 2)
                        for q in qk_steps:
                            q(2)
                        if last and ch == 1 and jt >= 1:
                            emit_wo_half((jt - 1) // 2, (jt - 1) % 2)
                if hp == 0 and ch == 0:
                    emit_v_chunk(NT - 1, 1)
                for q in qk_steps:
                    q(2 * NC)   # drain any remainder

                def make_tail(av_step=av_step, a0=a0, a1=a1, hp=hp, ch=ch):
                    def run():
                        av_step(NT - 2)
                        av_step(NT - 1)
                        normalize(a0, 0, hp, ch)
                        normalize(a1, 1, hp, ch)
                    return run

                pending_tail = make_tail()

        pending_tail()
        emit_wo_half(3, 1)
        emit_wo(range(NT // 2, NT))


_CACHED = {}
def build_program():
    if "nc" in _CACHED:
        return _CACHED["nc"]
    nc = bacc.Bacc("TRN2", target_bir_lowering=False, debug=False, num_devices=8)
    x_d = nc.dram_tensor("x", [T, C], F32, kind="ExternalInput").ap()
    wq_d = nc.dram_tensor("w_qkv", [C, 3 * C], F32, kind="ExternalInput").ap()
    wo_d = nc.dram_tensor("w_o", [C, C], F32, kind="ExternalInput").ap()
    out_d = nc.dram_tensor("out", [T, C], F32, kind="ExternalOutput").ap()
    with tile.TileContext(nc) as tc:
        attention_kernel(tc, out_d, x_d, wq_d, wo_d)
    nc.compile()
    _CACHED["nc"] = nc
    return nc


def kernel(x, w_qkv, w_o, _trace=False, _trace_cores=None):
    nc = build_program()
    x = np.ascontiguousarray(np.asarray(x, dtype=np.float32))
    w_qkv = np.ascontiguousarray(np.asarray(w_qkv, dtype=np.float32))
    w_o = np.ascontiguousarray(np.asarray(w_o, dtype=np.float32))
    bs = x.shape[0]
    in_maps = [
        {"x": x[b].reshape(T, C), "w_qkv": w_qkv, "w_o": w_o} for b in range(bs)
    ]
    res = bass_utils.run_bass_kernel_spmd(
        nc, in_maps, core_ids=list(range(bs)), trace=_trace,
        trace_cores=_trace_cores,
    )
    out = np.stack([res.results[b]["out"].reshape(32, 32, C) for b in range(bs)])
    if _trace:
        return out, res
    return out


# revision 37
# speedup vs baseline: 1.0178x; 1.0178x over previous
"""Trainium2 Bass kernel for nn_Attention_36644660969693.

Multi-head attention block: x[8,32,32,768] -> qkv -> 12-head attention -> wo.
Sharding: data-parallel over batch, one image (1024 tokens) per NeuronCore.

Per-core layout strategy (T=1024 tokens, C=768, 12 heads, hd=64):
  - all inputs arrive via gpsimd (SWDGE) casting DMAs straight into fp16
    SBUF tiles, column-sliced in priority order: x, pair-0 q/k columns,
    v columns, remaining q/k columns, w_o
  - xT[c,t] via PE transpose of x (fp16)
  - qkT[f,t] = w_qkv tile-stationary @ xT; head h lands at partition
    (h*64)%128 of f-tile h//2, so a head PAIR occupies the two partition
    halves of one tile -> 2-head row-packed score matmuls
  - v[t,f] natural orientation, stored per pair as [v_even | ones | v_odd]
    (192 cols); the AV stationary [128,128] slice for the even head is
    [v|1] and for the odd head [1|v], so each AV matmul also emits the
    softmax row-sums in the complementary 64 output partitions for free;
    v is computed just-in-time inside pair 0's loop
  - scoresT[j,i] per head = kT-tile-stationary @ qT (K=64, two heads packed
    into disjoint PE row groups), exp on ScalarE reads PSUM directly with
    the 1/8 scale fused, output fp16 to SBUF
  - AV accumulates v_pad.T @ expT over key tiles; 3 of 4 head/chunk
    accumulations pipeline inside the score/exp loop (lagging exp), the 4th
    is deferred into the next pair's loop so ScalarE never waits at a pair
    boundary
  - normalize = full-partition fast-reciprocal of the row-sum half +
    64-partition swap DMA + elementwise multiply
  - out[t,:] = attn_T-tile-stationary @ w_o (natural layout, DMA straight
    out)
All matmuls run in fp16 (1 cycle/row on PE) with fp32 PSUM accumulation.
ScalarE exp (~17us/head-pair) is the pipeline bottleneck; PE work is
interleaved under it.
"""

import numpy as np

import concourse.bass as bass
import concourse.tile as tile
from concourse import bacc, mybir
from concourse import bass_utils
from concourse import masks

P = 128          # partitions
T = 1024         # tokens per image
C = 768          # model dim
NT = T // P      # 8 token tiles
NC = C // P      # 6 channel tiles
NH = 12          # heads
HD = 64          # head dim
NPAIR = NH // 2  # 6 head pairs
VPW = 192        # v_pad pair block width: [v_even(64) | ones(64) | v_odd(64)]
SCALE = HD ** -0.5
F32 = mybir.dt.float32
F16 = mybir.dt.float16
EXP = mybir.ActivationFunctionType.Exp


def attention_kernel(tc, out_d, x_d, wq_d, wo_d):
    nc = tc.nc
    from contextlib import ExitStack

    with ExitStack() as ctx:
        const_pool = ctx.enter_context(tc.tile_pool(name="const", bufs=1))
        persist = ctx.enter_context(tc.tile_pool(name="persist", bufs=1))
        opool = ctx.enter_context(tc.tile_pool(name="ot", bufs=2))

        identh = const_pool.tile([P, P], F16, tag="identh")
        masks.make_identity(nc, identh[:])

        xT = persist.tile([P, NC * T], F16, tag="xT")        # [c, t] blocks
        wq = persist.tile([P, NC * 2304], F16, tag="wq")     # [c, f] blocks
        qkT = persist.tile([P, 12 * T], F16, tag="qkT")      # [f, t] blocks
        vpad = persist.tile([P, NT * NPAIR * VPW], F16, tag="vpad")
        aT = persist.tile([P, NC * T], F16, tag="aT")        # [c, t] blocks
        wo_sb = persist.tile([P, NC * C], F16, tag="wo")     # [c, c'] blocks

        # ones blocks of v_pad: cols 64:128 of each 192-col pair block
        ones_ap = vpad[:].rearrange(
            "p (blk w) -> p blk w", w=VPW
        )[:, :, HD: 2 * HD]
        nc.vector.memset(ones_ap, 1.0)

        # ---- input DMAs: gpsimd SWDGE casts f32->f16 in flight ----
        # priority order: x, pair-0 q/k columns, v columns, bulk q/k, w_o
        qkcol = [[768, 2], [1, 128]]     # cols 0:128 and 768:896
        bulkcol = [[768, 2], [1, 640]]   # cols 128:768 and 896:1536

        def wq_col_dma(ct, pattern, off):
            dst = wq[:, ct * 2304 + off: (ct + 1) * 2304]
            dst = bass.AP(
                tensor=dst.tensor, offset=dst.offset, ap=[dst.ap[0]] + pattern)
            s = wq_d[ct * P:(ct + 1) * P, off:2304]
            src = bass.AP(
                tensor=s.tensor, offset=s.offset, ap=[s.ap[0]] + pattern)
            nc.gpsimd.dma_start(dst, src)

        prep_ctx = ExitStack()
        pp_prep = prep_ctx.enter_context(
            tc.tile_pool(name="pprep", bufs=2, space="PSUM"))

        with tc.tile_pool(name="sx", bufs=2) as stage_x:
            for tb in range(2):
                xh = stage_x.tile([P, 4 * C], F16, tag="xh", name="xh")
                src = x_d[tb * 512:(tb + 1) * 512, :].rearrange(
                    "(k p) c -> p k c", p=P
                )
                nc.gpsimd.dma_start(
                    xh[:].rearrange("p (k c) -> p k c", k=4), src
                )
                for ct in range(NC):
                    ps = pp_prep.tile([P, 512], F16, tag="tr", name="ps_tr")
                    for k in range(4):
                        nc.tensor.transpose(
                            ps[:, k * P:(k + 1) * P],
                            xh[:, k * C + ct * P: k * C + ct * P + P],
                            identh[:],
                        )
                    dst_off = ct * T + tb * 512
                    nc.vector.tensor_copy(xT[:, dst_off: dst_off + 512], ps[:])

        for ct in range(NC):
            wq_col_dma(ct, qkcol, 0)          # pair-0 q/k columns
        for ct in range(NC):
            wq_col_dma(ct, [[1, 768]], 1536)  # v columns
        for ct in range(NC):
            wq_col_dma(ct, bulkcol, 128)      # remaining q/k columns
        for ct in range(NC):
            nc.gpsimd.dma_start(wo_sb[:, ct * C:(ct + 1) * C],
                                wo_d[ct * P:(ct + 1) * P, :])

        qk_psum = {"pool": pp_prep, "tag": "mm"}

        def make_qk_emitter(ft):
            """Returns step(n): emits n accumulation matmuls of the qkT
            f-tile computation, so the work interleaves finely with the
            score stream instead of blocking it."""
            pool, tag = qk_psum["pool"], qk_psum["tag"]
            st = {"ch": 0, "ct": 0, "ps": None}

            def step(n):
                for _ in range(n):
                    ch, ct = st["ch"], st["ct"]
                    if ch >= 2:
                        return
                    if ct == 0:
                        st["ps"] = pool.tile([P, 512], F32, tag=tag, name="ps_qk")
                    nc.tensor.matmul(
                        st["ps"][:],
                        wq[:, ct * 2304 + ft * P: ct * 2304 + ft * P + P],
                        xT[:, ct * T + ch * 512: ct * T + ch * 512 + 512],
                        start=(ct == 0),
                        stop=(ct == NC - 1),
                    )
                    if ct == NC - 1:
                        nc.vector.tensor_copy(
                            qkT[:, ft * T + ch * 512: ft * T + ch * 512 + 512],
                            st["ps"][:])
                        st["ch"], st["ct"] = ch + 1, 0
                    else:
                        st["ct"] = ct + 1
            return step

        def emit_qk_tile(ft):
            make_qk_emitter(ft)(2 * NC)

        emit_qk_tile(0)
        emit_qk_tile(6)
        prep_ctx.close()

        # ---- attention, one head pair at a time ----
        epool = ctx.enter_context(tc.tile_pool(name="E", bufs=2))
        rpool = ctx.enter_context(tc.tile_pool(name="recip", bufs=2))
        pp_s = ctx.enter_context(tc.tile_pool(name="pps", bufs=2, space="PSUM"))
        pp_av = ctx.enter_context(tc.tile_pool(name="ppav", bufs=4, space="PSUM"))
        qk_psum["pool"], qk_psum["tag"] = pp_av, "av"

        wo_state = {}

        def emit_wo_half(tt, half):
            """Half of one output t-tile projection (6 matmuls)."""
            if half == 0:
                po1 = pp_av.tile([P, 512], F32, tag="av", name="po1")
                wo_state[tt] = po1
                for ct in range(NC):
                    lhsT = aT[:, ct * T + tt * P: ct * T + tt * P + P]
                    nc.tensor.matmul(po1[:], lhsT, wo_sb[:, ct * C: ct * C + 512],
                                     start=(ct == 0), stop=(ct == NC - 1))
            else:
                po1 = wo_state.pop(tt)
                po2 = pp_av.tile([P, 512], F32, tag="av", name="po2")
                for ct in range(NC):
                    lhsT = aT[:, ct * T + tt * P: ct * T + tt * P + P]
                    nc.tensor.matmul(po2[:, :256], lhsT,
                                     wo_sb[:, ct * C + 512: ct * C + C],
                                     start=(ct == 0), stop=(ct == NC - 1))
                ot = opool.tile([P, C], F32, tag="ot", name="ot")
                nc.scalar.copy(ot[:, 0:512], po1[:])
                nc.vector.tensor_copy(ot[:, 512:C], po2[:, :256])
                nc.sync.dma_start(out_d[tt * P:(tt + 1) * P, :], ot[:])

        def emit_wo(tts):
            for tt in tts:
                emit_wo_half(tt, 0)
                emit_wo_half(tt, 1)

        def vslice(jt, hp, h):
            """[128,128] AV stationary: even head [v|1], odd head [1|v]."""
            base = jt * NPAIR * VPW + hp * VPW + (0 if h == 0 else HD)
            return vpad[:, base: base + P]

        def emit_v_chunk(tt, chunk):
            """One 6-matmul chunk of v[t-tile tt] (chunk 0: heads 0-7,
            chunk 1: heads 8-11)."""
            for (foff, fw) in ((0, 512), (512, 256))[chunk:chunk + 1]:
                ps = pp_av.tile([P, 512], F32, tag="av", name="ps_v")
                for ct in range(NC):
                    nc.tensor.matmul(
                        ps[:, :fw],
                        xT[:, ct * T + tt * P: ct * T + tt * P + P],
                        wq[:, ct * 2304 + 1536 + foff: ct * 2304 + 1536 + foff + fw],
                        start=(ct == 0),
                        stop=(ct == NC - 1),
                    )
                npr = fw // 128
                src = ps[:, :fw].rearrange("p (m two d) -> p m two d", two=2, d=HD)
                base = tt * NPAIR * VPW + (foff // 128) * VPW
                dst = vpad[:, base: base + npr * VPW].rearrange(
                    "p (m blk) -> p m blk", blk=VPW
                )
                nc.vector.tensor_copy(dst[:, :, 0:HD], src[:, :, 0, :])
                nc.vector.tensor_copy(dst[:, :, 2 * HD:VPW], src[:, :, 1, :])

        def normalize(a, h, hp, ch):
            r = rpool.tile([P, 512], F32, tag="r", name=f"r{h}{ch}")
            r2 = rpool.tile([P, 512], F32, tag="r2", name=f"r2{h}{ch}")
            dst = aT[:, hp * T + ch * 512: hp * T + ch * 512 + 512]
            # full-partition approx reciprocal (custom DVE op needs base
            # partition 0); the non-rowsum half of r is garbage, never read
            nc.vector.reciprocal_approx_fast(r[:, :], a[:, :])
            if h == 0:
                nc.sync.dma_start(r2[0:HD, :], r[HD:P, :])
                nc.vector.tensor_mul(dst[0:HD, :], a[0:HD, :], r2[0:HD, :])
            else:
                nc.sync.dma_start(r2[HD:P, :], r[0:HD, :])
                nc.vector.tensor_mul(dst[HD:P, :], a[HD:P, :], r2[HD:P, :])

        def eoff(jt, ch, h):
            return jt * 2048 + ch * T + h * 512

        def emit_scores_exp(hp, jt, ch, E):
            """Both packed heads' scores for one i-chunk into ONE 2-bank
            psum tile (forces the row-group pair to issue back-to-back),
            then a single exp over the pair."""
            qblk = hp * T
            kblk = (6 + hp) * T
            s = pp_s.tile([P, T], F32, tag="s", name="s")
            nc.tensor.matmul(
                s[:, 0:512],
                qkT[0:HD, kblk + jt * P: kblk + jt * P + P],
                qkT[0:HD, qblk + ch * 512: qblk + ch * 512 + 512],
                start=True, stop=True,
            )
            nc.tensor.matmul(
                s[:, 512:1024],
                qkT[HD:P, kblk + jt * P: kblk + jt * P + P],
                qkT[HD:P, qblk + ch * 512: qblk + ch * 512 + 512],
                start=True, stop=True,
            )
            nc.scalar.activation(E[:, eoff(jt, ch, 0): eoff(jt, ch, 0) + T],
                                 s[:], EXP, scale=SCALE)

        pending_tail = None   # previous half-pass: final AV steps + normalizes

        for hp in range(NPAIR):
            E = epool.tile([P, NT * 2048], F16, tag="E", name="E")
            last = hp == NPAIR - 1

            for ch in range(2):
                a0 = pp_av.tile([P, 512], F32, tag="av", name=f"a0c{ch}")
                a1 = pp_av.tile([P, 512], F32, tag="av", name=f"a1c{ch}")

                def av_step(jt, ch=ch, a0=a0, a1=a1, hp=hp, E=E):
                    for a, h in ((a0, 0), (a1, 1)):
                        nc.tensor.matmul(
                            a[:],
                            vslice(jt, hp, h),
                            E[:, eoff(jt, ch, h): eoff(jt, ch, h) + 512],
                            start=(jt == 0),
                            stop=(jt == NT - 1),
                        )

                # fine-grained filler schedule for this half-pass
                if hp == 0 and ch == 1:
                    qk_steps = [make_qk_emitter(1), make_qk_emitter(7)]
                elif 0 < hp < NPAIR - 1:
                    qk_steps = [make_qk_emitter(hp + 1 if ch == 0 else 6 + hp + 1)]
                else:
                    qk_steps = []

                for jtp in range(0, NT, 2):
                    # two adjacent score-pair groups: their stationaries sit
                    # in disjoint PE row groups, so weight loads pre-overlap
                    for jt in (jtp, jtp + 1):
                        emit_scores_exp(hp, jt, ch, E)
                        if jt == 0 and pending_tail is not None:
                            pending_tail()
                            pending_tail = None
                        if hp == 0 and ch == 0:
                            emit_v_chunk(jt, 0)
                            if jt >= 1:
                                emit_v_chunk(jt - 1, 1)
                    for jt in (jtp, jtp + 1):
                        if jt >= 2:
                            av_step(jt - 2)
                        for q in qk_steps:
                            q(2)
                        if last and ch == 1 and jt >= 1:
                            emit_wo_half((jt - 1) // 2, (jt - 1) % 2)
                if hp == 0 and ch == 0:
                    emit_v_chunk(NT - 1, 1)
                for q in qk_steps:
                    q(2 * NC)   # drain any remainder

                def make_tail(av_step=av_step, a0=a0, a1=a1, hp=hp, ch=ch):
                    def run():
                        av_step(NT - 2)
                        av_step(NT - 1)
                        normalize(a0, 0, hp, ch)
                        normalize(a1, 1, hp, ch)
                    return run

                pending_tail = make_tail()

        pending_tail()
        emit_wo_half(3, 1)
        emit_wo(range(NT // 2, NT))


_CACHED = {}
def build_program():
    if "nc" in _CACHED:
        return _CACHED["nc"]
    nc = bacc.Bacc("TRN2", target_bir_lowering=False, debug=False, num_devices=8)
    x_d = nc.dram_tensor("x", [T, C], F32, kind="ExternalInput").ap()
    wq_d = nc.dram_tensor("w_qkv", [C, 3 * C], F32, kind="ExternalInput").ap()
    wo_d = nc.dram_tensor("w_o", [C, C], F32, kind="ExternalInput").ap()
    out_d = nc.dram_tensor("out", [T, C], F32, kind="ExternalOutput").ap()
    with tile.TileContext(nc) as tc:
        attention_kernel(tc, out_d, x_d, wq_d, wo_d)
    nc.compile()
    _CACHED["nc"] = nc
    return nc


def kernel(x, w_qkv, w_o, _trace=False, _trace_cores=None):
    nc = build_program()
    x = np.ascontiguousarray(np.asarray(x, dtype=np.float32))
    w_qkv = np.ascontiguousarray(np.asarray(w_qkv, dtype=np.float32))
    w_o = np.ascontiguousarray(np.asarray(w_o, dtype=np.float32))
    bs = x.shape[0]
    in_maps = [
        {"x": x[b].reshape(T, C), "w_qkv": w_qkv, "w_o": w_o} for b in range(bs)
    ]
    res = bass_utils.run_bass_kernel_spmd(
        nc, in_maps, core_ids=list(range(bs)), trace=_trace,
        trace_cores=_trace_cores,
    )
    out = np.stack([res.results[b]["out"].reshape(32, 32, C) for b in range(bs)])
    if _trace:
        return out, res
    return out
